# revision 5
# baseline (speedup 1.0000x reference)
"""Trainium2 Bass kernel for the CT-metrics pairwise MLP scorer.

Computes, for M_aug [N,D] and Q [M,D] (N=M=512, D=1024):
    diff2[n,m,:] = (M_aug[n]-Q[m])**2
    cost[n,m]    = diff2.sum(-1)
    d[n,m]       = -(MLP(diff2[n,m,:]) + b3)      (D->512->256->1, leaky relu)
    bw           = softmax(d, axis=0)
    score[m]     = sum_n cost*bw,  score_fg[m] = sum_{n<N_fg} cost*bw

Sharding: N axis split across 8 cores (64 rows each). Each core computes
partial column sums S1 = sum_n exp(d), S1fg, Sc' = sum_n exp(d)*(||Mn||^2
- 2 Mn.Q^T) and Scfg' (flash-softmax; logits are O(1) so no max shift).
Host combine:  score = Sc'/S1 + ||Q||^2, score_fg = Scfg'/S1 + ||Q||^2*S1fg/S1.

Device algorithm (fp8 DoubleRow formulation):
  diff2 = q^2 - 2mq + m^2.  The n-independent W1 @ q^2 + b1 term is folded
  into a precomputed Btilde[h,m] (computed once on device from host-packed
  fp8 q^2, stored fp8 in duplicate planes, injected into each row's L1 PSUM
  via a 0.5*identity-pair DoubleRow matmul).  Per row only
      u[d,m] = q[d,m]*(-2 m[n,d]) + m[n,d]^2
  is materialized - one DVE/GPSIMD tensor_scalar op (bf16 in, f32 scalars,
  fp8 out; DVE 2x_2p mode).  All MLP layers are fp8e4 DoubleRow matmuls
  (0.5 cyc/row); b1/b2 are injected as fp8 rank-2 DoubleRow matmuls so the
  h1/h2 activations are bias-free and read 2-bank PSUM pairs in single ACT
  instructions.  The PE stream is software-pipelined (iter k runs L1(k),
  L2(k-1), L3(k-2)) so it never waits on ACT.  A 40-column tail of each
  h-activation is offloaded to DVE (scalar-mult + tensor max) to balance
  ACT vs DVE.  Inputs arrive as 6 DMAs ordered so the Btilde prologue and
  first u-chunks overlap the remaining input transfers.
"""

from contextlib import ExitStack

import numpy as np

import concourse.bass as bass
import concourse.tile as tile
from concourse import bacc, mybir
from concourse.bass_utils import run_bass_kernel_spmd

N_CORES = 8
N, M, D, H = 512, 512, 1024, 512
K2 = H // 2  # 256
NL = N // N_CORES  # 64 rows per core
DC, HC, KC, MC = D // 128, H // 128, K2 // 128, M // 128  # 8, 4, 2, 4
DP, HP = DC // 2, HC // 2  # DoubleRow plane-pairs: 4, 2
NEG_SLOPE = 0.01
W_OFF = 40  # h-activation columns (per PSUM bank) offloaded ACT -> DVE

F32 = mybir.dt.float32
BF16 = mybir.dt.bfloat16
FP8 = mybir.dt.float8e4
AF = mybir.ActivationFunctionType
DR = mybir.MatmulPerfMode.DoubleRow

# which engine produces each u chunk: 'v' = DVE, 'p' = GPSIMD
U_SPLIT = "vvpvpvpv"

# ---- packed fp8 tensor column offsets ----
_W10 = 0                       # W1 DoubleRow layout [128, DP*2*H]
_W20 = _W10 + DP * 2 * H       # W2 DoubleRow layout [128, HP*2*K2]
_W30 = _W20 + HP * 2 * K2      # W3 DoubleRow layout [128, 2*1]
_Q20 = _W30 + 2                # (Q^T)^2 DoubleRow layout [128, DP*2*M]
_IH0 = _Q20 + DP * 2 * M       # 0.5*I identity pair [128, 2*128]
_BB20 = _IH0 + 2 * 128         # b2/2 pairs (part 0) [1, KC*2*128]
_BB10 = _BB20 + KC * 2 * 128   # b1/2 pairs (part 0) [1, HC*2*128]
_O80 = _BB10 + HC * 2 * 128    # ones pair fp8 (part 0) [1, 2*M]
_P8_COLS = _O80 + 2 * M

# ---- f32 scalars tensor (s1, s2) ----
_S10 = 0                       # -2*M^T  [128, DC*NL]
_S20 = _S10 + DC * NL          # (M^T)^2 [128, DC*NL]
_PFS_COLS = _S20 + DC * NL

# ---- bf16 remainder tensor (mt, ones) ----
_MT0 = 0                       # -M^T chunks [128, DC*NL]
_OB0 = _MT0 + DC * NL          # ones row bf16 (part 0) [1, M]
_PBR_COLS = _OB0 + M

# ---- f32 remainder tensor ----
_MN0 = 0                       # 0.5*||Mn||^2 row (partition 0) [1, NL]
_FG0 = _MN0 + NL               # fg mask row tiled MC times (part 0)
_ON0 = _FG0 + MC * NL          # ones row f32 (part 0) [1, 128]
_B30 = _ON0 + 128              # -b3 column [128, 1]
_PFR_COLS = _B30 + 1


def emit_body(nc, tc, ctx, p8_sb, pq_sb, pfs_sb, pbr_sb, pfr_sb, stats):
    def pool(name, bufs, space="SBUF"):
        return ctx.enter_context(tc.tile_pool(name=name, bufs=bufs, space=space))

    consts = ctx.enter_context(tc.tile_pool(name="consts2", bufs=1))
    diffp = pool("diffp", 2)
    h1p = pool("h1p", 2)
    h2p = pool("h2p", 2)
    ep = pool("ep", 1)
    # one shared 2-bank PSUM pool: per row holds l1a, l1b, p2 (rotating)
    ps2b = pool("ps2b", 3, "PSUM")
    psd = pool("psd", 1, "PSUM")  # [128, MC, NL] logits / cost psum

    w1_8 = p8_sb[:, _W10:_W20].rearrange("p (i j h) -> p i j h", i=DP, j=2)
    w2_8 = p8_sb[:, _W20:_W30].rearrange("p (i j k) -> p i j k", i=HP, j=2)
    w3_8 = p8_sb[:, _W30:_W30 + 2].rearrange("p (j o) -> p j o", j=2)
    q2_8 = p8_sb[:, _Q20:_IH0].rearrange("p (i j m) -> p i j m", i=DP, j=2)
    ih_8 = p8_sb[:, _IH0:_BB20].rearrange("p (j q) -> p j q", j=2)
    bb2_8 = p8_sb[0:1, _BB20:_BB10].rearrange("p (k j q) -> p k j q", k=KC, j=2)
    bb1_8 = p8_sb[0:1, _BB10:_O80].rearrange("p (h j q) -> p h j q", h=HC, j=2)
    o8_2 = p8_sb[0:1, _O80:_O80 + 2 * M].rearrange("p (j m) -> p j m", j=2)

    qt_bf = pq_sb.rearrange("p (c m) -> p c m", c=DC)
    s1f = pfs_sb[:, _S10:_S20].rearrange("p (c n) -> p c n", c=DC)
    s2f = pfs_sb[:, _S20:_PFS_COLS].rearrange("p (c n) -> p c n", c=DC)
    mt_bf = pbr_sb[:, _MT0:_OB0].rearrange("p (c n) -> p c n", c=DC)
    mn2h = pfr_sb[0:1, _MN0:_MN0 + NL]
    fg_row = pfr_sb[0:1, _FG0:_FG0 + MC * NL]
    onesf = pfr_sb[0:1, _ON0:_ON0 + 128]
    b3n = pfr_sb[:, _B30:_B30 + 1]

    # ================= prologue =================
    # Btilde[h, m] = W1 @ q^2 + b1, stored fp8 twice (planes for DoubleRow
    # injection via the 0.5*I identity pair).  Depends only on the p8 DMA.
    btz = consts.tile([128, HC, 2, M], FP8, tag="btz")
    for hpair in range(2):
        bt_ps = ps2b.tile([128, 2, M], F32, tag="psx", name=f"bt{hpair}")
        for sub in range(2):
            hc = hpair * 2 + sub
            for i in range(DP):
                nc.tensor.matmul(bt_ps[:, sub, :],
                                 w1_8[:, i, :, hc * 128:(hc + 1) * 128],
                                 q2_8[:, i, :, :],
                                 start=(i == 0), stop=False, perf_mode=DR)
            nc.tensor.matmul(bt_ps[:, sub, :], bb1_8[:, hc, :, :], o8_2,
                             start=False, stop=True, perf_mode=DR)
        for j in range(2):
            nc.scalar.activation(btz[:, hpair * 2:hpair * 2 + 2, j, :],
                                 bt_ps[:], AF.Copy)

    # fg mask broadcast to all partitions
    mask_ps = ps2b.tile([128, MC * NL], F32, tag="psx", name="maskp")
    nc.tensor.matmul(mask_ps[:], onesf, fg_row, start=True, stop=True)
    mask_bc = consts.tile([128, MC, NL], F32, tag="mask_bc")
    nc.vector.tensor_copy(mask_bc[:], mask_ps[:].rearrange(
        "p (c n) -> p c n", c=MC))

    # cost'[n, m] = ||Mn||^2 - 2 Mn.Qm  (x2 applied below; ||Q||^2 on host)
    g_t = psd.tile([128, MC, NL], F32, tag="psd", name="g_t")
    for mc in range(MC):
        for dc in range(DC):
            nc.tensor.matmul(g_t[:, mc, :],
                             qt_bf[:, dc, mc * 128:(mc + 1) * 128],
                             mt_bf[:, dc, :],
                             start=(dc == 0), stop=False)
        nc.tensor.matmul(g_t[:, mc, :], onesf, mn2h, start=False, stop=True)
    cost_t = consts.tile([128, MC, NL], F32, tag="cost_t")
    nc.vector.tensor_scalar_mul(cost_t[:], g_t[:], 2.0)

    d_ps = psd.tile([128, MC, NL], F32, tag="psd", name="d_ps")

    # ================= main loop over local rows =================
    # Software-pipelined so the in-order PE stream never waits on ACT:
    # iteration k emits  u(k), L1(k), h1(k), L2(k-1), h2(k-1), L3(k-2).
    MULT = mybir.AluOpType.mult
    ADD = mybir.AluOpType.add
    MAX = mybir.AluOpType.max
    MA = M - W_OFF  # ACT covers [0:MA], DVE covers [MA:M] of each bank
    h1_hist = {}
    h2_hist = {}

    def dve_leaky(dst, src):
        # dst (fp8 SBUF) = leaky_relu(src (f32 PSUM)); two DVE ops, the
        # scaled branch written first then max'd in place against src.
        nc.vector.tensor_scalar_mul(dst, src, NEG_SLOPE)
        nc.vector.tensor_tensor(dst, src, dst, op=MAX)

    def emit_u_l1_h1(n):
        u = diffp.tile([128, DC, M], FP8, tag="u")
        for dc in range(DC):
            eng = nc.vector if U_SPLIT[dc] == "v" else nc.gpsimd
            eng.tensor_scalar(u[:, dc, :], qt_bf[:, dc, :],
                              s1f[:, dc, n:n + 1], s2f[:, dc, n:n + 1],
                              op0=MULT, op1=ADD)
        h1 = h1p.tile([128, HC, M], FP8, tag="h1")
        h1_hist[n] = h1
        for hpair in range(2):
            p1 = ps2b.tile([128, 2, M], F32, tag="psx", name=f"p1_{hpair}")
            for sub in range(2):
                hc = hpair * 2 + sub
                for i in range(DP):
                    nc.tensor.matmul(p1[:, sub, :],
                                     w1_8[:, i, :, hc * 128:(hc + 1) * 128],
                                     u[:, 2 * i:2 * i + 2, :],
                                     start=(i == 0), stop=False, perf_mode=DR)
                nc.tensor.matmul(p1[:, sub, :], ih_8,
                                 btz[:, hc, :, :],
                                 start=False, stop=True, perf_mode=DR)
            nc.scalar.activation(h1[:, hpair * 2:hpair * 2 + 2, 0:MA],
                                 p1[:, :, 0:MA], AF.Lrelu, alpha=NEG_SLOPE)
            dve_leaky(h1[:, hpair * 2:hpair * 2 + 2, MA:M], p1[:, :, MA:M])

    def emit_l2_h2(n):
        h1 = h1_hist.pop(n)
        p2 = ps2b.tile([128, KC, M], F32, tag="psx", name="p2")
        for kc in range(KC):
            for i in range(HP):
                nc.tensor.matmul(p2[:, kc, :],
                                 w2_8[:, i, :, kc * 128:(kc + 1) * 128],
                                 h1[:, 2 * i:2 * i + 2, :],
                                 start=(i == 0), stop=False, perf_mode=DR)
            nc.tensor.matmul(p2[:, kc, :], bb2_8[:, kc, :, :], o8_2,
                             start=False, stop=True, perf_mode=DR)
        h2 = h2p.tile([128, KC, M], FP8, tag="h2")
        h2_hist[n] = h2
        nc.scalar.activation(h2[:, :, 0:MA], p2[:, :, 0:MA],
                             AF.Lrelu, alpha=NEG_SLOPE)
        dve_leaky(h2[:, :, MA:M], p2[:, :, MA:M])

    def emit_l3(n):
        h2 = h2_hist.pop(n)
        for mc in range(MC):
            nc.tensor.matmul(d_ps[:, mc, n:n + 1],
                             h2[:, :, mc * 128:(mc + 1) * 128], w3_8,
                             start=True, stop=True, perf_mode=DR)

    for n in range(NL):
        emit_u_l1_h1(n)
        if n >= 1:
            emit_l2_h2(n - 1)
        if n >= 2:
            emit_l3(n - 2)
    emit_l2_h2(NL - 1)
    emit_l3(NL - 2)
    emit_l3(NL - 1)

    # ================= epilogue =================
    # DVE handles e/w/wfg sums; GPSIMD handles the efg path in parallel.
    e_t = ep.tile([128, MC, NL], F32, tag="e_t")
    nc.scalar.activation(e_t[:], d_ps[:], AF.Exp, bias=b3n, scale=-1.0)
    stats_sb = consts.tile([128, 4, MC], F32, tag="stats_sb")
    w_t = ep.tile([128, MC, NL], F32, tag="w_t")
    nc.vector.tensor_mul(w_t[:], e_t[:], cost_t[:])
    nc.vector.tensor_reduce(stats_sb[:, 0, :], e_t[:],
                            axis=mybir.AxisListType.X, op=ADD)
    efg_t = ep.tile([128, MC, NL], F32, tag="efg_t")
    nc.gpsimd.tensor_mul(efg_t[:], e_t[:], mask_bc[:])
    wfg_t = ep.tile([128, MC, NL], F32, tag="wfg_t")
    nc.vector.tensor_mul(wfg_t[:], w_t[:], mask_bc[:])
    nc.vector.tensor_reduce(stats_sb[:, 2, :], w_t[:],
                            axis=mybir.AxisListType.X, op=ADD)
    nc.vector.tensor_reduce(stats_sb[:, 1, :], efg_t[:],
                            axis=mybir.AxisListType.X, op=ADD)
    nc.vector.tensor_reduce(stats_sb[:, 3, :], wfg_t[:],
                            axis=mybir.AxisListType.X, op=ADD)
    nc.sync.dma_start(stats[:], stats_sb[:])


def build_program():
    nc = bacc.Bacc("TRN2", target_bir_lowering=False, debug=False,
                   num_devices=N_CORES)
    p8 = nc.dram_tensor("p8", [128, _P8_COLS], FP8, kind="ExternalInput").ap()
    pq1 = nc.dram_tensor("pq1", [128, DC * M // 2], BF16,
                         kind="ExternalInput").ap()
    pfs = nc.dram_tensor("pfs", [128, _PFS_COLS], F32,
                         kind="ExternalInput").ap()
    pq2 = nc.dram_tensor("pq2", [128, DC * M // 2], BF16,
                         kind="ExternalInput").ap()
    pbr = nc.dram_tensor("pbr", [128, _PBR_COLS], BF16,
                         kind="ExternalInput").ap()
    pfr = nc.dram_tensor("pfr", [128, _PFR_COLS], F32,
                         kind="ExternalInput").ap()
    stats = nc.dram_tensor("stats", [128, 4, MC], F32,
                           kind="ExternalOutput").ap()

    with tile.TileContext(nc) as tc, ExitStack() as ctx:
        consts = ctx.enter_context(tc.tile_pool(name="consts", bufs=1))
        p8_sb = consts.tile([128, _P8_COLS], FP8, tag="p8_sb")
        nc.sync.dma_start(p8_sb[:], p8[:])
        pq_sb = consts.tile([128, DC * M], BF16, tag="pq_sb")
        nc.sync.dma_start(pq_sb[:, 0:DC * M // 2], pq1[:])
        pfs_sb = consts.tile([128, _PFS_COLS], F32, tag="pfs_sb")
        nc.sync.dma_start(pfs_sb[:], pfs[:])
        nc.sync.dma_start(pq_sb[:, DC * M // 2:DC * M], pq2[:])
        pbr_sb = consts.tile([128, _PBR_COLS], BF16, tag="pbr_sb")
        nc.sync.dma_start(pbr_sb[:], pbr[:])
        pfr_sb = consts.tile([128, _PFR_COLS], F32, tag="pfr_sb")
        nc.sync.dma_start(pfr_sb[:], pfr[:])
        emit_body(nc, tc, ctx, p8_sb, pq_sb, pfs_sb, pbr_sb, pfr_sb, stats)

    nc.compile()
    return nc


def shard_inputs(M_aug, Q, W1, b1, W2, b2, W3, b3, N_fg):
    """Host-side layout prep. Returns per-core input maps."""
    import ml_dtypes
    f = np.float32
    bf = ml_dtypes.bfloat16
    f8 = ml_dtypes.float8_e4m3
    M_aug = np.asarray(M_aug, f)
    Q = np.asarray(Q, f)
    W1 = np.asarray(W1, f)
    W2 = np.asarray(W2, f)
    W3 = np.asarray(W3, f)
    b1 = np.asarray(b1, f)
    b2 = np.asarray(b2, f)
    b3 = np.asarray(b3, f)
    nfg = int(N_fg)

    def part_major(a2d, chunks):  # [C*128, F] -> [128, C*F]
        cdim, fdim = a2d.shape
        assert cdim == chunks * 128
        return np.ascontiguousarray(
            a2d.reshape(chunks, 128, fdim).transpose(1, 0, 2)).reshape(128, -1)

    def dr_layout(a2d, pairs):  # [2*pairs*128, F] -> [128, pairs*2*F]
        return part_major(a2d, 2 * pairs)

    def half_pairs(b, chunks):  # [C*128] -> [1, C*2*128] of b/2 twice
        hp = np.zeros((chunks, 2, 128), f)
        hp[:, 0, :] = 0.5 * b.reshape(chunks, 128)
        hp[:, 1, :] = 0.5 * b.reshape(chunks, 128)
        return hp.reshape(-1)

    # ---- fp8 packed tensor (shared across cores) ----
    p8_v = np.zeros((128, _P8_COLS), f8)
    p8_v[:, _W10:_W20] = dr_layout(W1.T, DP).astype(f8)
    p8_v[:, _W20:_W30] = dr_layout(W2.T, HP).astype(f8)
    p8_v[:, _W30:_W30 + 2] = dr_layout(W3.reshape(K2, 1), 1).astype(f8)
    qt_bf_full = Q.T.astype(bf)
    q2 = (qt_bf_full.astype(f) ** 2).astype(bf).astype(f)
    p8_v[:, _Q20:_IH0] = dr_layout(q2, DP).astype(f8)
    ih = np.zeros((2, 128, 128), f)
    ih[0] = 0.5 * np.eye(128)
    ih[1] = 0.5 * np.eye(128)
    p8_v[:, _IH0:_BB20] = ih.transpose(1, 0, 2).reshape(128, -1).astype(f8)
    p8_v[0, _BB20:_BB10] = half_pairs(b2, KC).astype(f8)
    p8_v[0, _BB10:_O80] = half_pairs(b1, HC).astype(f8)
    p8_v[0, _O80:_O80 + 2 * M] = np.ones(2 * M, f).astype(f8)
    p8_v = np.ascontiguousarray(p8_v)

    qt_pm = part_major(Q.T, DC).astype(bf)
    pq1_v = np.ascontiguousarray(qt_pm[:, :DC * M // 2])
    pq2_v = np.ascontiguousarray(qt_pm[:, DC * M // 2:])

    pfr_base = np.zeros((128, _PFR_COLS), f)
    pfr_base[0, _ON0:_ON0 + 128] = 1.0
    pfr_base[:, _B30] = -float(b3[0])

    in_maps = []
    for c in range(N_CORES):
        rows = slice(c * NL, (c + 1) * NL)
        Mrows = M_aug[rows]
        pfs_v = np.zeros((128, _PFS_COLS), f)
        pfs_v[:, _S10:_S20] = part_major(-2.0 * Mrows.T, DC)
        pfs_v[:, _S20:_PFS_COLS] = part_major(Mrows.T ** 2, DC)
        pbr_v = np.zeros((128, _PBR_COLS), bf)
        pbr_v[:, _MT0:_OB0] = part_major(-Mrows.T, DC).astype(bf)
        pbr_v[0, _OB0:_OB0 + M] = np.ones(M, f).astype(bf)
        pfr_v = pfr_base.copy()
        pfr_v[0, _MN0:_MN0 + NL] = 0.5 * (Mrows ** 2).sum(-1)
        gidx = np.arange(c * NL, (c + 1) * NL)
        pfr_v[0, _FG0:_FG0 + MC * NL] = np.tile((gidx < nfg).astype(f), MC)
        in_maps.append({"p8": p8_v,
                        "pq1": pq1_v, "pq2": pq2_v,
                        "pfs": np.ascontiguousarray(pfs_v),
                        "pbr": np.ascontiguousarray(pbr_v),
                        "pfr": np.ascontiguousarray(pfr_v)})
    return in_maps


def combine(stats_list, Q):
    """stats_list: per-core [128, 4, MC] arrays -> (score, score_fg)."""
    st = np.stack([
        np.asarray(s, np.float64).transpose(1, 2, 0).reshape(4, M)
        for s in stats_list
    ])  # [C, 4, M]
    S1 = st[:, 0].sum(0)
    S1fg = st[:, 1].sum(0)
    Sc = st[:, 2].sum(0)
    Scfg = st[:, 3].sum(0)
    qn2 = (np.asarray(Q, np.float64) ** 2).sum(-1)
    score = Sc / S1 + qn2
    score_fg = Scfg / S1 + qn2 * (S1fg / S1)
    return score.astype(np.float32), score_fg.astype(np.float32)


_PROGRAM_CACHE = {}


def run(trace=False, **inputs):
    if "prog" not in _PROGRAM_CACHE:
        _PROGRAM_CACHE["prog"] = build_program()
    nc = _PROGRAM_CACHE["prog"]
    in_maps = shard_inputs(**inputs)
    res = run_bass_kernel_spmd(nc, in_maps, list(range(N_CORES)), trace=trace)
    outs = combine([res.results[c]["stats"] for c in range(N_CORES)],
                   inputs["Q"])
    return outs, res


def kernel(**inputs):
    outs, _ = run(trace=False, **inputs)
    return outs
